# revision 39
# baseline (speedup 1.0000x reference)
"""Trainium2 Bass kernel for BANDNN-style packed-molecule energy model.

Reference computation: four per-row MLPs (one per interaction type), each
row's scalar energy segment-summed per molecule; four per-molecule energies
added -> (512, 1).

Strategy (8 NeuronCores, data-parallel over molecules):
 - Each core owns 64 contiguous molecules; host slices the packed rows of all
   four types accordingly, transposes to feature-major (F, N) layout, and
   zero-pads N so all cores share one NEFF.
 - Per 512-row tile the MLP runs feature-major on the PE (h^T = W^T @ x^T).
   Matmul operands are bf16 (PE full rate; fp32r streams at half rate on
   TRN2); PSUM accumulation and everything downstream of the last matmul
   (e = h3.W4 + b4, segment sums) stays fp32.
 - Per-row energies land in DRAM; a per-molecule indirect-DMA gather +
   iota<len mask + multiply-reduce produces the 64 per-molecule sums on each
   core; host concatenates the 8 outputs.
"""

import os as _os
import sys

if "/opt/trn_rl_repo" not in sys.path:
    sys.path.insert(0, "/opt/trn_rl_repo")

import numpy as np

B = 512            # molecules
NCORES = 8
MPC = B // NCORES  # molecules per core
TILE = 512         # matmul moving-operand width (one fp32 PSUM bank)
CHW = 8192         # x^T DMA chunk width (16 tiles; also the L4/est batch)

# nonbonds last: its phase-B segment reduction is the kernel tail
TYPES = (("bonds", 13), ("angles", 19), ("dihedrals", 25), ("nonbonds", 6))

IOTA_W = 512

MM_DTYPE = _os.environ.get("KERNEL_MM_DTYPE", "bf16")  # "bf16" | "f32r"



def _round_up(x, m):
    return -(-x // m) * m


def _round_fp32r(a):
    """Round fp32 to float32r precision (11 explicit mantissa bits)."""
    u = np.ascontiguousarray(a, dtype=np.float32).view(np.uint32).astype(np.uint64)
    u = (u + 0x7FF + ((u >> 12) & 1)) >> 12 << 12
    return np.minimum(u, 0xFFFFFFFF).astype(np.uint32).view(np.float32)


def _const_layout(cfg):
    """Column layouts: weights tensor (matmul dtype) and scalars tensor (f32)."""
    woff = {}
    soff = {}
    wc = 0
    sc = 0
    for name, _F, _NP, _RM in cfg:
        d = {}
        for w in ("w1", "w2a", "w2b", "w3a", "w3b"):
            d[w] = wc
            wc += 128
        d["w4"] = wc
        wc += 1
        d["w4p"] = wc
        wc += 16 * 16
        woff[name] = d
        s = {}
        for k in ("b1", "b2a", "b2b", "b3", "b4", "lens", "lensb"):
            s[k] = sc
            sc += 1
        soff[name] = s
    soff["iota"] = sc
    sc += IOTA_W
    return woff, wc, soff, sc


def _build(cfg, split_late):
    """Build + bacc-compile the Bass kernel for the given per-type config."""
    from contextlib import ExitStack

    import concourse.bacc as bacc
    import concourse.mybir as mybir
    import concourse.tile as tile
    from concourse.bass import IndirectOffsetOnAxis
    from concourse.tile import add_dep_helper

    f32 = mybir.dt.float32
    i32 = mybir.dt.int32
    mmdt = mybir.dt.bfloat16 if MM_DTYPE == "bf16" else mybir.dt.float32r
    RELU = mybir.ActivationFunctionType.Relu
    IDENT = mybir.ActivationFunctionType.Identity
    ADD = mybir.AluOpType.add
    MAX = mybir.AluOpType.max

    woff, nw, soff, ns = _const_layout(cfg)

    nc = bacc.Bacc(
        "TRN2",
        target_bir_lowering=False,
        debug=False,
        enable_asserts=False,
        num_devices=NCORES,
    )

    wts_ap = nc.dram_tensor("wts", (128, nw), mmdt, kind="ExternalInput").ap()
    scl_ap = nc.dram_tensor("scl", (128, ns), f32, kind="ExternalInput").ap()
    xt_aps = {}
    ebuf_aps = {}
    starts_aps = {}
    for name, F, NP, _RM in cfg:
        xt_aps[name] = nc.dram_tensor(
            f"{name}_xt", (128, NP // 4), mmdt, kind="ExternalInput"
        ).ap()
        ebuf_aps[name] = nc.dram_tensor(
            f"{name}_ebuf", (NP, 1), f32, kind="Internal"
        ).ap()
        starts_aps[name] = nc.dram_tensor(
            f"{name}_starts", (MPC, 1), i32, kind="ExternalInput"
        ).ap()
    last_name = cfg[-1][0]
    if split_late:
        startsb_ap = nc.dram_tensor(
            f"{last_name}_startsb", (32, 1), i32, kind="ExternalInput"
        ).ap()
    out_ap = nc.dram_tensor("out", (MPC, 1), f32, kind="ExternalOutput").ap()

    with tile.TileContext(nc) as tc, ExitStack() as ctx:
        cpool = ctx.enter_context(tc.tile_pool(name="consts", bufs=1))
        warmt = cpool.tile([128, TILE], mmdt, tag="warm")
        nc.vector.memset(warmt[:], 0.125)
        wtile = cpool.tile([128, nw], mmdt, tag="wts")
        nc.sync.dma_start(wtile[:], wts_ap[:])
        stile = cpool.tile([128, ns], f32, tag="scl")
        nc.sync.dma_start(stile[:], scl_ap[:])

        xpools = {
            name: ctx.enter_context(
                tc.tile_pool(name=f"x_{name}", bufs=min(-(-NP // CHW), 4))
            )
            for name, _F, NP, _RM in cfg
        }
        h1p = ctx.enter_context(tc.tile_pool(name="h1", bufs=9))
        h2ap = ctx.enter_context(tc.tile_pool(name="h2a", bufs=9))
        h2bp = ctx.enter_context(tc.tile_pool(name="h2b", bufs=9))
        h3p = ctx.enter_context(tc.tile_pool(name="h3", bufs=9))
        ep = ctx.enter_context(tc.tile_pool(name="estage", bufs=2))
        ph13 = ctx.enter_context(tc.tile_pool(name="ph13", bufs=4, space="PSUM"))
        ph2 = ctx.enter_context(tc.tile_pool(name="ph2", bufs=3, space="PSUM"))
        pep = ctx.enter_context(tc.tile_pool(name="pe", bufs=1, space="PSUM"))
        segp = ctx.enter_context(tc.tile_pool(name="seg", bufs=1))

        # HAM warmup: ~24 dense back-to-back matmuls (~10us cold) trip the
        # PE clock gate to 8/8 before the real pipeline (which has micro-gaps
        # that never satisfy the "sustained busy" window) begins.
        warm = pep.tile([8, TILE], f32, tag="pe")
        for k in range(16):
            nc.tensor.matmul(
                warm[0:8, :], warmt[:, 0:8], warmt[:, :],
                start=(k == 0), stop=(k == 15),
            )

        masks = {}
        starts_tiles = {}
        for name, _F, _NP, RM in cfg:
            so = soff[name]
            st = segp.tile([MPC, 1], i32, tag=f"starts_{name}")
            nc.sync.dma_start(st[:], starts_aps[name][:])
            starts_tiles[name] = st
            mk = segp.tile([MPC, RM], f32, tag=f"mask_{name}")
            nc.vector.tensor_scalar(
                mk[:],
                stile[0:MPC, soff["iota"] : soff["iota"] + RM],
                stile[0:MPC, so["lens"] : so["lens"] + 1],
                None,
                op0=mybir.AluOpType.is_lt,
            )
            masks[name] = mk
            if split_late and name == last_name:
                mkb = segp.tile([32, RM], f32, tag="maskb")
                nc.vector.tensor_scalar(
                    mkb[:],
                    stile[MPC - 32 : MPC, soff["iota"] : soff["iota"] + RM],
                    stile[MPC - 32 : MPC, so["lensb"] : so["lensb"] + 1],
                    None,
                    op0=mybir.AluOpType.is_lt,
                )
                masks["_late"] = mkb

        acc = None
        for name, F, NP, RM in cfg:
            wo = woff[name]
            so = soff[name]
            w1g = [
                wtile[32 * g : 32 * g + F, wo["w1"] : wo["w1"] + 128]
                for g in range(4)
            ]
            w2a = wtile[:, wo["w2a"] : wo["w2a"] + 128]
            w2b = wtile[:, wo["w2b"] : wo["w2b"] + 128]
            w3a = wtile[:, wo["w3a"] : wo["w3a"] + 128]
            w3b = wtile[:, wo["w3b"] : wo["w3b"] + 128]
            w4 = wtile[:, wo["w4"] : wo["w4"] + 1]
            b1 = stile[:, so["b1"] : so["b1"] + 1]
            b2a = stile[:, so["b2a"] : so["b2a"] + 1]
            b2b = stile[:, so["b2b"] : so["b2b"] + 1]
            b3 = stile[:, so["b3"] : so["b3"] + 1]
            b4 = stile[:, so["b4"] : so["b4"] + 1]
            xt_ap = xt_aps[name]
            ebuf_ap = ebuf_aps[name]
            ebuf_writes = []

            chunk_plan = [(c0, min(CHW, NP - c0)) for c0 in range(0, NP, CHW)]
            for c0, w in chunk_plan:
                nt = w // TILE
                xt = xpools[name].tile([128, w // 4], mmdt)
                nc.sync.dma_start(xt[:], xt_ap[:, c0 // 4 : (c0 + w) // 4])
                est = ep.tile([nt, TILE], f32, tag="est")
                # one PSUM bank collects the chunk's nt (1,512) energy rows
                pe = pep.tile([16, TILE], f32, tag="pe")
                # layer-major over groups of 4 tiles: PE runs 4+ same-layer
                # matmuls back-to-back while ACT/DVE copies chase, instead of
                # stalling on each tile's copy chain.
                for g0 in range(0, nt, 8):
                    gn = min(8, nt - g0)
                    p1s = []
                    h1s = []
                    for sub0 in range(0, gn, 4):
                        sn = min(4, gn - sub0)
                        qq = (g0 + sub0) // 4
                        subp = []
                        for g in range(sn):
                            rhs = xt[32 * g : 32 * g + F, qq * TILE : (qq + 1) * TILE]
                            p1 = ph13.tile([128, TILE], f32, tag="p13")
                            nc.tensor.matmul(
                                p1[:], w1g[g], rhs, start=True, stop=True,
                                tile_position=(32 * g, 0),
                            )
                            subp.append(p1)
                        for g in range(sn):
                            h1 = h1p.tile([128, TILE], mmdt)
                            nc.scalar.activation(
                                h1[:], subp[g][:], RELU, bias=b1, scale=1.0
                            )
                            h1s.append(h1)
                    h2s = []
                    for k in range(gn):
                        p2a = ph2.tile([128, TILE], f32, tag="p2")
                        nc.tensor.matmul(p2a[:], w2a, h1s[k][:], start=True, stop=True)
                        h2a = h2ap.tile([128, TILE], mmdt)
                        nc.vector.tensor_scalar(h2a[:], p2a[:], b2a, 0.0, op0=ADD, op1=MAX)
                        p2b = ph2.tile([128, TILE], f32, tag="p2")
                        nc.tensor.matmul(p2b[:], w2b, h1s[k][:], start=True, stop=True)
                        h2b = h2bp.tile([128, TILE], mmdt)
                        nc.vector.tensor_scalar(h2b[:], p2b[:], b2b, 0.0, op0=ADD, op1=MAX)
                        h2s.append((h2a, h2b))
                    h3s = []
                    for k in range(gn):
                        h2a, h2b = h2s[k]
                        p3 = ph13.tile([128, TILE], f32, tag="p13")
                        nc.tensor.matmul(p3[:], w3a, h2a[:], start=True, stop=False)
                        nc.tensor.matmul(p3[:], w3b, h2b[:], start=False, stop=True)
                        h3 = h3p.tile([128, TILE], mmdt)
                        nc.scalar.activation(h3[:], p3[:], RELU, bias=b3, scale=1.0)
                        h3s.append(h3)
                    for k in range(gn):
                        j = g0 + k
                        w4p = wtile[:, wo["w4p"] + 16 * j : wo["w4p"] + 16 * (j + 1)]
                        nc.tensor.matmul(
                            pe[0:16, :],
                            w4p,
                            h3s[k][:],
                            start=(j == 0),
                            stop=(j == nt - 1),
                        )
                nc.scalar.activation(
                    est[:, :], pe[0:nt, :], IDENT, bias=b4[0:nt, :], scale=1.0
                )
                ebuf_writes.append(
                    nc.sync.dma_start(ebuf_ap[c0 : c0 + w, :], est[:]).ins
                )

            # ---- per-molecule segment reduction for this type ----
            # For the last type, molecules 32-63 (whose gather window can
            # touch the final chunk) are gathered separately so the main
            # gather only waits on all-but-the-last chunk.
            split = split_late and name == last_name
            starts_sb = starts_tiles[name]
            mask = masks[name]
            eg = segp.tile([MPC, RM], f32, tag=f"eg_{name}")
            g = nc.gpsimd.indirect_dma_start(
                out=eg[:],
                out_offset=None,
                in_=ebuf_ap[:, :],
                in_offset=IndirectOffsetOnAxis(ap=starts_sb[:, :1], axis=0),
            )
            for winst in ebuf_writes[: -1 if split else None]:
                add_dep_helper(g.ins, winst, sync=True, reason="ebuf RAW")

            scratch = segp.tile([MPC, RM], f32, tag=f"scr_{name}")
            acc_t = segp.tile([MPC, 1], f32, tag=f"acct_{name}")
            nc.vector.tensor_tensor(
                out=scratch[:], in0=eg[:], in1=mask[:], op=mybir.AluOpType.mult
            )
            nc.vector.reduce_sum(acc_t[:], scratch[:], axis=mybir.AxisListType.X)
            if acc is None:
                acc = acc_t
            else:
                acc_new = segp.tile([MPC, 1], f32, tag=f"acc_{name}")
                nc.vector.tensor_tensor(
                    out=acc_new[:], in0=acc[:], in1=acc_t[:], op=ADD
                )
                acc = acc_new
            if split:
                startsb_sb = segp.tile([32, 1], i32, tag="startsb")
                nc.sync.dma_start(startsb_sb[:], startsb_ap[:])
                egb = segp.tile([32, RM], f32, tag="egb")
                gb = nc.gpsimd.indirect_dma_start(
                    out=egb[:],
                    out_offset=None,
                    in_=ebuf_ap[:, :],
                    in_offset=IndirectOffsetOnAxis(ap=startsb_sb[:, :1], axis=0),
                )
                for winst in ebuf_writes:
                    add_dep_helper(gb.ins, winst, sync=True, reason="ebuf RAW b")
                scrb = segp.tile([32, RM], f32, tag="scrb")
                accb = segp.tile([MPC, 1], f32, tag="accb")
                nc.vector.memset(accb[:], 0.0)
                nc.vector.tensor_tensor(
                    out=scrb[:], in0=egb[:], in1=masks["_late"][:, :],
                    op=mybir.AluOpType.mult,
                )
                nc.vector.reduce_sum(
                    accb[MPC - 32 : MPC, :], scrb[:], axis=mybir.AxisListType.X
                )
                acc_fin = segp.tile([MPC, 1], f32, tag="accfin")
                nc.vector.tensor_tensor(
                    out=acc_fin[:], in0=acc[:], in1=accb[:], op=ADD
                )
                acc = acc_fin

        nc.sync.dma_start(out_ap[:, :], acc[:])

    nc.compile()
    return nc


_CACHE = {}


def _get_nc(cfg, split_late):
    key = (cfg, split_late)
    if key not in _CACHE:
        _CACHE[key] = _build(cfg, split_late)
    return _CACHE[key]


def _mm_np_dtype():
    if MM_DTYPE == "bf16":
        import ml_dtypes

        return np.dtype(ml_dtypes.bfloat16)
    return np.dtype(np.float32)


def _to_mm(a):
    if MM_DTYPE == "bf16":
        return np.ascontiguousarray(a).astype(_mm_np_dtype())
    return _round_fp32r(a)


def _prep(inputs):
    """Host-side sharding: per-core input dicts + per-type config."""
    cfg = []
    per_type = {}
    for name, F in TYPES:
        x = np.ascontiguousarray(np.asarray(inputs[name + "_x"], dtype=np.float32))
        seg = np.asarray(inputs[name + "_seg"], dtype=np.int64)
        assert x.shape[1] == F
        counts = np.bincount(seg, minlength=B)
        bounds = np.searchsorted(seg, np.arange(0, B + 1, MPC))
        rows_pc = np.diff(bounds)
        RM = _round_up(int(counts.max()), 4)
        assert RM <= IOTA_W
        NP = _round_up(int(rows_pc.max()) + RM, 4 * TILE)
        mol_starts = np.searchsorted(seg, np.arange(B))
        per_type[name] = dict(
            x=x, counts=counts, bounds=bounds, mol_starts=mol_starts, NP=NP, RM=RM
        )
        cfg.append((name, F, NP, RM))
    cfg = tuple(cfg)

    woff, nw, soff, ns = _const_layout(cfg)
    mmdt = _mm_np_dtype()

    # late-molecule split eligibility for the last type: molecule MPC-32 of
    # every core must start early enough that its gather window stays clear
    # of the final chunk
    lname = TYPES[-1][0]
    lpt = per_type[lname]
    last_w = lpt["NP"] - ((lpt["NP"] - 1) // CHW) * CHW
    cut = lpt["NP"] - last_w
    split_late = True
    for c in range(NCORES):
        s = int(lpt["bounds"][c])
        st32 = int(lpt["mol_starts"][c * MPC + MPC - 32] - s)
        if st32 + lpt["RM"] > cut:
            split_late = False

    wts = np.zeros((128, nw), dtype=mmdt)
    scl = np.zeros((128, ns), dtype=np.float32)
    for name, F in TYPES:
        wo = woff[name]
        so = soff[name]
        params = inputs[name + "_params"]
        (W1, b1), (W2, b2), (W3, b3), (W4, b4) = [
            (np.asarray(w, np.float32), np.asarray(b, np.float32)) for w, b in params
        ]
        for g in range(4):
            wts[32 * g : 32 * g + F, wo["w1"] : wo["w1"] + 128] = _to_mm(W1)
        wts[:, wo["w2a"] : wo["w2a"] + 128] = _to_mm(W2[:, :128])
        wts[:, wo["w2b"] : wo["w2b"] + 128] = _to_mm(W2[:, 128:])
        wts[:, wo["w3a"] : wo["w3a"] + 128] = _to_mm(W3[:128, :])
        wts[:, wo["w3b"] : wo["w3b"] + 128] = _to_mm(W3[128:, :])
        wts[:, wo["w4"]] = _to_mm(W4[:, 0])
        for j in range(16):
            wts[:, wo["w4p"] + 16 * j + j] = _to_mm(W4[:, 0])
        scl[:, so["b1"]] = b1
        scl[:, so["b2a"]] = b2[:128]
        scl[:, so["b2b"]] = b2[128:]
        scl[:, so["b3"]] = b3
        scl[:, so["b4"]] = b4[0]
    scl[0:MPC, soff["iota"] : soff["iota"] + IOTA_W] = np.arange(
        IOTA_W, dtype=np.float32
    )[None, :]

    in_maps = []
    for c in range(NCORES):
        m = {"wts": wts, "scl": scl.copy()}
        for name, F in TYPES:
            pt = per_type[name]
            s, e = int(pt["bounds"][c]), int(pt["bounds"][c + 1])
            NPt = pt["NP"]
            xT = np.zeros((F, NPt), dtype=np.float32)
            xT[:, : e - s] = pt["x"][s:e].T
            x3 = xT.reshape(F, NPt // (4 * TILE), 4, TILE)
            fold = np.zeros((128, NPt // 4), dtype=mmdt)
            for g in range(4):
                fold[32 * g : 32 * g + F] = _to_mm(
                    x3[:, :, g, :].reshape(F, NPt // 4)
                )
            m[name + "_xt"] = fold
            so = soff[name]
            starts = (pt["mol_starts"][c * MPC : (c + 1) * MPC] - s).astype(np.int32)
            lens = pt["counts"][c * MPC : (c + 1) * MPC].astype(np.float32)
            if split_late and name == TYPES[-1][0]:
                m[name + "_startsb"] = starts[MPC - 32 :].reshape(32, 1).copy()
                m["scl"][MPC - 32 : MPC, so["lensb"]] = lens[MPC - 32 :]
                starts = starts.copy()
                lens = lens.copy()
                starts[MPC - 32 :] = 0
                lens[MPC - 32 :] = 0.0
            m[name + "_starts"] = starts.reshape(MPC, 1)
            m["scl"][0:MPC, so["lens"]] = lens
        in_maps.append(m)
    return cfg, split_late, in_maps


def kernel(**inputs) -> np.ndarray:
    from concourse import bass_utils

    cfg, split_late, in_maps = _prep(inputs)
    nc = _get_nc(cfg, split_late)
    res = bass_utils.run_bass_kernel_spmd(nc, in_maps, core_ids=list(range(NCORES)))
    return np.concatenate([res.results[c]["out"] for c in range(NCORES)], axis=0)


# revision 40
# speedup vs baseline: 1.0419x; 1.0419x over previous
"""Trainium2 Bass kernel for BANDNN-style packed-molecule energy model.

Reference computation: four per-row MLPs (one per interaction type), each
row's scalar energy segment-summed per molecule; four per-molecule energies
added -> (512, 1).

Strategy (8 NeuronCores, data-parallel over molecules):
 - Each core owns 64 contiguous molecules; host slices the packed rows of all
   four types accordingly, transposes to feature-major layout (row-tiles
   folded 4-up across 32-partition strips), and zero-pads so all cores share
   one NEFF.
 - Per 512-row tile the MLP runs feature-major on the PE (h^T = W^T @ x^T).
   Matmul operands are bf16 (PE full rate; fp32r streams at half rate on
   TRN2); PSUM accumulation and everything downstream of the last matmul
   (e = h3.W4 + b4, segment sums) stays fp32. Final rel err vs the fp32
   reference is ~5e-3 (KERNEL_MM_DTYPE=f32r gives ~1e-4 at ~2x the time).
 - Emission is layer-major over groups of 8 tiles so the PE sees dense
   same-layer matmul bursts (keeps the HAM clock gate at 2.4 GHz; per-tile
   emission leaves micro-gaps that pin the PE at the cold 1.2 GHz rate).
   L1 uses 4-way tile_position row packing (K = 6..25); the four L4s of
   consecutive tiles write disjoint partition rows of one PSUM bank via
   shifted-column W4 copies, batching the energy-row copy 16 tiles at a
   time.
 - Per-row energies land in DRAM; a per-molecule indirect-DMA gather +
   iota<len mask + multiply-reduce produces the 64 per-molecule sums on each
   core; host concatenates the 8 outputs. For the last (largest) type the
   final 32 molecules gather separately so the main gather clears before the
   last chunk finishes.
"""

import os as _os
import sys

if "/opt/trn_rl_repo" not in sys.path:
    sys.path.insert(0, "/opt/trn_rl_repo")

import numpy as np

B = 512            # molecules
NCORES = 8
MPC = B // NCORES  # molecules per core
TILE = 512         # matmul moving-operand width (one fp32 PSUM bank)
CHW = 8192         # x^T DMA chunk width (16 tiles; also the L4/est batch)

# nonbonds last: its phase-B segment reduction is the kernel tail
TYPES = (("bonds", 13), ("angles", 19), ("dihedrals", 25), ("nonbonds", 6))

IOTA_W = 512

MM_DTYPE = _os.environ.get("KERNEL_MM_DTYPE", "bf16")  # "bf16" | "f32r"



def _round_up(x, m):
    return -(-x // m) * m


def _round_fp32r(a):
    """Round fp32 to float32r precision (11 explicit mantissa bits)."""
    u = np.ascontiguousarray(a, dtype=np.float32).view(np.uint32).astype(np.uint64)
    u = (u + 0x7FF + ((u >> 12) & 1)) >> 12 << 12
    return np.minimum(u, 0xFFFFFFFF).astype(np.uint32).view(np.float32)


def _const_layout(cfg):
    """Column layouts: weights tensor (matmul dtype) and scalars tensor (f32)."""
    woff = {}
    soff = {}
    wc = 0
    sc = 0
    for name, _F, _NP, _RM in cfg:
        d = {}
        for w in ("w1", "w2a", "w2b", "w3a", "w3b"):
            d[w] = wc
            wc += 128
        d["w4"] = wc
        wc += 1
        d["w4p"] = wc
        wc += 16 * 16
        woff[name] = d
        s = {}
        for k in ("b1", "b2a", "b2b", "b3", "b4", "lens", "lensb"):
            s[k] = sc
            sc += 1
        soff[name] = s
    soff["iota"] = sc
    sc += IOTA_W
    return woff, wc, soff, sc


def _build(cfg, split_late):
    """Build + bacc-compile the Bass kernel for the given per-type config."""
    from contextlib import ExitStack

    import concourse.bacc as bacc
    import concourse.mybir as mybir
    import concourse.tile as tile
    from concourse.bass import IndirectOffsetOnAxis
    from concourse.tile import add_dep_helper

    f32 = mybir.dt.float32
    i32 = mybir.dt.int32
    mmdt = mybir.dt.bfloat16 if MM_DTYPE == "bf16" else mybir.dt.float32r
    RELU = mybir.ActivationFunctionType.Relu
    IDENT = mybir.ActivationFunctionType.Identity
    ADD = mybir.AluOpType.add
    MAX = mybir.AluOpType.max

    woff, nw, soff, ns = _const_layout(cfg)

    nc = bacc.Bacc(
        "TRN2",
        target_bir_lowering=False,
        debug=False,
        enable_asserts=False,
        num_devices=NCORES,
    )

    wts_ap = nc.dram_tensor("wts", (128, nw), mmdt, kind="ExternalInput").ap()
    scl_ap = nc.dram_tensor("scl", (128, ns), f32, kind="ExternalInput").ap()
    xt_aps = {}
    ebuf_aps = {}
    starts_aps = {}
    for name, F, NP, _RM in cfg:
        xt_aps[name] = nc.dram_tensor(
            f"{name}_xt", (128, NP // 4), mmdt, kind="ExternalInput"
        ).ap()
        ebuf_aps[name] = nc.dram_tensor(
            f"{name}_ebuf", (NP, 1), f32, kind="Internal"
        ).ap()
        starts_aps[name] = nc.dram_tensor(
            f"{name}_starts", (MPC, 1), i32, kind="ExternalInput"
        ).ap()
    last_name = cfg[-1][0]
    if split_late:
        startsb_ap = nc.dram_tensor(
            f"{last_name}_startsb", (32, 1), i32, kind="ExternalInput"
        ).ap()
    out_ap = nc.dram_tensor("out", (MPC, 1), f32, kind="ExternalOutput").ap()

    with tile.TileContext(nc) as tc, ExitStack() as ctx:
        cpool = ctx.enter_context(tc.tile_pool(name="consts", bufs=1))
        warmt = cpool.tile([128, TILE], mmdt, tag="warm")
        nc.vector.memset(warmt[:], 0.125)
        wtile = cpool.tile([128, nw], mmdt, tag="wts")
        nc.sync.dma_start(wtile[:], wts_ap[:])
        stile = cpool.tile([128, ns], f32, tag="scl")
        nc.sync.dma_start(stile[:], scl_ap[:])

        xpools = {
            name: ctx.enter_context(
                tc.tile_pool(name=f"x_{name}", bufs=min(-(-NP // CHW), 4))
            )
            for name, _F, NP, _RM in cfg
        }
        h1p = ctx.enter_context(tc.tile_pool(name="h1", bufs=9))
        h2ap = ctx.enter_context(tc.tile_pool(name="h2a", bufs=9))
        h2bp = ctx.enter_context(tc.tile_pool(name="h2b", bufs=9))
        h3p = ctx.enter_context(tc.tile_pool(name="h3", bufs=9))
        ep = ctx.enter_context(tc.tile_pool(name="estage", bufs=2))
        ph13 = ctx.enter_context(tc.tile_pool(name="ph13", bufs=4, space="PSUM"))
        ph2 = ctx.enter_context(tc.tile_pool(name="ph2", bufs=3, space="PSUM"))
        pep = ctx.enter_context(tc.tile_pool(name="pe", bufs=1, space="PSUM"))
        segp = ctx.enter_context(tc.tile_pool(name="seg", bufs=1))

        # HAM warmup: ~24 dense back-to-back matmuls (~10us cold) trip the
        # PE clock gate to 8/8 before the real pipeline (which has micro-gaps
        # that never satisfy the "sustained busy" window) begins.
        warm = pep.tile([8, TILE], f32, tag="pe")
        for k in range(16):
            nc.tensor.matmul(
                warm[0:8, :], warmt[:, 0:8], warmt[:, :],
                start=(k == 0), stop=(k == 15),
            )

        masks = {}
        starts_tiles = {}
        for name, _F, _NP, RM in cfg:
            so = soff[name]
            st = segp.tile([MPC, 1], i32, tag=f"starts_{name}")
            nc.sync.dma_start(st[:], starts_aps[name][:])
            starts_tiles[name] = st
            mk = segp.tile([MPC, RM], f32, tag=f"mask_{name}")
            nc.vector.tensor_scalar(
                mk[:],
                stile[0:MPC, soff["iota"] : soff["iota"] + RM],
                stile[0:MPC, so["lens"] : so["lens"] + 1],
                None,
                op0=mybir.AluOpType.is_lt,
            )
            masks[name] = mk
            if split_late and name == last_name:
                mkb = segp.tile([32, RM], f32, tag="maskb")
                nc.vector.tensor_scalar(
                    mkb[:],
                    stile[MPC - 32 : MPC, soff["iota"] : soff["iota"] + RM],
                    stile[MPC - 32 : MPC, so["lensb"] : so["lensb"] + 1],
                    None,
                    op0=mybir.AluOpType.is_lt,
                )
                masks["_late"] = mkb

        acc = None
        for name, F, NP, RM in cfg:
            wo = woff[name]
            so = soff[name]
            w1g = [
                wtile[32 * g : 32 * g + F, wo["w1"] : wo["w1"] + 128]
                for g in range(4)
            ]
            w2a = wtile[:, wo["w2a"] : wo["w2a"] + 128]
            w2b = wtile[:, wo["w2b"] : wo["w2b"] + 128]
            w3a = wtile[:, wo["w3a"] : wo["w3a"] + 128]
            w3b = wtile[:, wo["w3b"] : wo["w3b"] + 128]
            w4 = wtile[:, wo["w4"] : wo["w4"] + 1]
            b1 = stile[:, so["b1"] : so["b1"] + 1]
            b2a = stile[:, so["b2a"] : so["b2a"] + 1]
            b2b = stile[:, so["b2b"] : so["b2b"] + 1]
            b3 = stile[:, so["b3"] : so["b3"] + 1]
            b4 = stile[:, so["b4"] : so["b4"] + 1]
            xt_ap = xt_aps[name]
            ebuf_ap = ebuf_aps[name]
            ebuf_writes = []

            chunk_plan = [(c0, min(CHW, NP - c0)) for c0 in range(0, NP, CHW)]
            for c0, w in chunk_plan:
                nt = w // TILE
                xt = xpools[name].tile([128, w // 4], mmdt)
                nc.sync.dma_start(xt[:], xt_ap[:, c0 // 4 : (c0 + w) // 4])
                est = ep.tile([nt, TILE], f32, tag="est")
                # one PSUM bank collects the chunk's nt (1,512) energy rows
                pe = pep.tile([16, TILE], f32, tag="pe")
                # layer-major over groups of 4 tiles: PE runs 4+ same-layer
                # matmuls back-to-back while ACT/DVE copies chase, instead of
                # stalling on each tile's copy chain.
                for g0 in range(0, nt, 8):
                    gn = min(8, nt - g0)
                    p1s = []
                    h1s = []
                    for sub0 in range(0, gn, 4):
                        sn = min(4, gn - sub0)
                        qq = (g0 + sub0) // 4
                        subp = []
                        for g in range(sn):
                            rhs = xt[32 * g : 32 * g + F, qq * TILE : (qq + 1) * TILE]
                            p1 = ph13.tile([128, TILE], f32, tag="p13")
                            nc.tensor.matmul(
                                p1[:], w1g[g], rhs, start=True, stop=True,
                                tile_position=(32 * g, 0),
                            )
                            subp.append(p1)
                        for g in range(sn):
                            h1 = h1p.tile([128, TILE], mmdt)
                            nc.scalar.activation(
                                h1[:], subp[g][:], RELU, bias=b1, scale=1.0
                            )
                            h1s.append(h1)
                    h2s = []
                    for k in range(gn):
                        p2a = ph2.tile([128, TILE], f32, tag="p2")
                        nc.tensor.matmul(p2a[:], w2a, h1s[k][:], start=True, stop=True)
                        h2a = h2ap.tile([128, TILE], mmdt)
                        nc.vector.tensor_scalar(h2a[:], p2a[:], b2a, 0.0, op0=ADD, op1=MAX)
                        p2b = ph2.tile([128, TILE], f32, tag="p2")
                        nc.tensor.matmul(p2b[:], w2b, h1s[k][:], start=True, stop=True)
                        h2b = h2bp.tile([128, TILE], mmdt)
                        nc.vector.tensor_scalar(h2b[:], p2b[:], b2b, 0.0, op0=ADD, op1=MAX)
                        h2s.append((h2a, h2b))
                    h3s = []
                    for k in range(gn):
                        h2a, h2b = h2s[k]
                        p3 = ph13.tile([128, TILE], f32, tag="p13")
                        nc.tensor.matmul(p3[:], w3a, h2a[:], start=True, stop=False)
                        nc.tensor.matmul(p3[:], w3b, h2b[:], start=False, stop=True)
                        h3 = h3p.tile([128, TILE], mmdt)
                        nc.scalar.activation(h3[:], p3[:], RELU, bias=b3, scale=1.0)
                        h3s.append(h3)
                    for k in range(gn):
                        j = g0 + k
                        w4p = wtile[:, wo["w4p"] + 16 * j : wo["w4p"] + 16 * (j + 1)]
                        nc.tensor.matmul(
                            pe[0:16, :],
                            w4p,
                            h3s[k][:],
                            start=(j == 0),
                            stop=(j == nt - 1),
                        )
                nc.scalar.activation(
                    est[:, :], pe[0:nt, :], IDENT, bias=b4[0:nt, :], scale=1.0
                )
                ebuf_writes.append(
                    nc.sync.dma_start(ebuf_ap[c0 : c0 + w, :], est[:]).ins
                )

            # ---- per-molecule segment reduction for this type ----
            # For the last type, molecules 32-63 (whose gather window can
            # touch the final chunk) are gathered separately so the main
            # gather only waits on all-but-the-last chunk.
            split = split_late and name == last_name
            starts_sb = starts_tiles[name]
            mask = masks[name]
            eg = segp.tile([MPC, RM], f32, tag=f"eg_{name}")
            g = nc.gpsimd.indirect_dma_start(
                out=eg[:],
                out_offset=None,
                in_=ebuf_ap[:, :],
                in_offset=IndirectOffsetOnAxis(ap=starts_sb[:, :1], axis=0),
            )
            for winst in ebuf_writes[: -1 if split else None]:
                add_dep_helper(g.ins, winst, sync=True, reason="ebuf RAW")

            scratch = segp.tile([MPC, RM], f32, tag=f"scr_{name}")
            acc_t = segp.tile([MPC, 1], f32, tag=f"acct_{name}")
            nc.vector.tensor_tensor(
                out=scratch[:], in0=eg[:], in1=mask[:], op=mybir.AluOpType.mult
            )
            nc.vector.reduce_sum(acc_t[:], scratch[:], axis=mybir.AxisListType.X)
            if acc is None:
                acc = acc_t
            else:
                acc_new = segp.tile([MPC, 1], f32, tag=f"acc_{name}")
                nc.vector.tensor_tensor(
                    out=acc_new[:], in0=acc[:], in1=acc_t[:], op=ADD
                )
                acc = acc_new
            if split:
                startsb_sb = segp.tile([32, 1], i32, tag="startsb")
                nc.sync.dma_start(startsb_sb[:], startsb_ap[:])
                egb = segp.tile([32, RM], f32, tag="egb")
                gb = nc.gpsimd.indirect_dma_start(
                    out=egb[:],
                    out_offset=None,
                    in_=ebuf_ap[:, :],
                    in_offset=IndirectOffsetOnAxis(ap=startsb_sb[:, :1], axis=0),
                )
                for winst in ebuf_writes:
                    add_dep_helper(gb.ins, winst, sync=True, reason="ebuf RAW b")
                scrb = segp.tile([32, RM], f32, tag="scrb")
                accb = segp.tile([MPC, 1], f32, tag="accb")
                nc.vector.memset(accb[:], 0.0)
                nc.vector.tensor_tensor(
                    out=scrb[:], in0=egb[:], in1=masks["_late"][:, :],
                    op=mybir.AluOpType.mult,
                )
                nc.vector.reduce_sum(
                    accb[MPC - 32 : MPC, :], scrb[:], axis=mybir.AxisListType.X
                )
                acc_fin = segp.tile([MPC, 1], f32, tag="accfin")
                nc.vector.tensor_tensor(
                    out=acc_fin[:], in0=acc[:], in1=accb[:], op=ADD
                )
                acc = acc_fin

        nc.sync.dma_start(out_ap[:, :], acc[:])

    nc.compile()
    return nc


_CACHE = {}


def _get_nc(cfg, split_late):
    key = (cfg, split_late)
    if key not in _CACHE:
        _CACHE[key] = _build(cfg, split_late)
    return _CACHE[key]


def _mm_np_dtype():
    if MM_DTYPE == "bf16":
        import ml_dtypes

        return np.dtype(ml_dtypes.bfloat16)
    return np.dtype(np.float32)


def _to_mm(a):
    if MM_DTYPE == "bf16":
        return np.ascontiguousarray(a).astype(_mm_np_dtype())
    return _round_fp32r(a)


def _prep(inputs):
    """Host-side sharding: per-core input dicts + per-type config."""
    cfg = []
    per_type = {}
    for name, F in TYPES:
        x = np.ascontiguousarray(np.asarray(inputs[name + "_x"], dtype=np.float32))
        seg = np.asarray(inputs[name + "_seg"], dtype=np.int64)
        assert x.shape[1] == F
        counts = np.bincount(seg, minlength=B)
        bounds = np.searchsorted(seg, np.arange(0, B + 1, MPC))
        rows_pc = np.diff(bounds)
        RM = _round_up(int(counts.max()), 4)
        assert RM <= IOTA_W
        NP = _round_up(int(rows_pc.max()) + RM, 4 * TILE)
        mol_starts = np.searchsorted(seg, np.arange(B))
        per_type[name] = dict(
            x=x, counts=counts, bounds=bounds, mol_starts=mol_starts, NP=NP, RM=RM
        )
        cfg.append((name, F, NP, RM))
    cfg = tuple(cfg)

    woff, nw, soff, ns = _const_layout(cfg)
    mmdt = _mm_np_dtype()

    # late-molecule split eligibility for the last type: molecule MPC-32 of
    # every core must start early enough that its gather window stays clear
    # of the final chunk
    lname = TYPES[-1][0]
    lpt = per_type[lname]
    last_w = lpt["NP"] - ((lpt["NP"] - 1) // CHW) * CHW
    cut = lpt["NP"] - last_w
    split_late = True
    for c in range(NCORES):
        s = int(lpt["bounds"][c])
        st32 = int(lpt["mol_starts"][c * MPC + MPC - 32] - s)
        if st32 + lpt["RM"] > cut:
            split_late = False

    wts = np.zeros((128, nw), dtype=mmdt)
    scl = np.zeros((128, ns), dtype=np.float32)
    for name, F in TYPES:
        wo = woff[name]
        so = soff[name]
        params = inputs[name + "_params"]
        (W1, b1), (W2, b2), (W3, b3), (W4, b4) = [
            (np.asarray(w, np.float32), np.asarray(b, np.float32)) for w, b in params
        ]
        for g in range(4):
            wts[32 * g : 32 * g + F, wo["w1"] : wo["w1"] + 128] = _to_mm(W1)
        wts[:, wo["w2a"] : wo["w2a"] + 128] = _to_mm(W2[:, :128])
        wts[:, wo["w2b"] : wo["w2b"] + 128] = _to_mm(W2[:, 128:])
        wts[:, wo["w3a"] : wo["w3a"] + 128] = _to_mm(W3[:128, :])
        wts[:, wo["w3b"] : wo["w3b"] + 128] = _to_mm(W3[128:, :])
        wts[:, wo["w4"]] = _to_mm(W4[:, 0])
        for j in range(16):
            wts[:, wo["w4p"] + 16 * j + j] = _to_mm(W4[:, 0])
        scl[:, so["b1"]] = b1
        scl[:, so["b2a"]] = b2[:128]
        scl[:, so["b2b"]] = b2[128:]
        scl[:, so["b3"]] = b3
        scl[:, so["b4"]] = b4[0]
    scl[0:MPC, soff["iota"] : soff["iota"] + IOTA_W] = np.arange(
        IOTA_W, dtype=np.float32
    )[None, :]

    in_maps = []
    for c in range(NCORES):
        m = {"wts": wts, "scl": scl.copy()}
        for name, F in TYPES:
            pt = per_type[name]
            s, e = int(pt["bounds"][c]), int(pt["bounds"][c + 1])
            NPt = pt["NP"]
            xT = np.zeros((F, NPt), dtype=np.float32)
            xT[:, : e - s] = pt["x"][s:e].T
            x3 = xT.reshape(F, NPt // (4 * TILE), 4, TILE)
            fold = np.zeros((128, NPt // 4), dtype=mmdt)
            for g in range(4):
                fold[32 * g : 32 * g + F] = _to_mm(
                    x3[:, :, g, :].reshape(F, NPt // 4)
                )
            m[name + "_xt"] = fold
            so = soff[name]
            starts = (pt["mol_starts"][c * MPC : (c + 1) * MPC] - s).astype(np.int32)
            lens = pt["counts"][c * MPC : (c + 1) * MPC].astype(np.float32)
            if split_late and name == TYPES[-1][0]:
                m[name + "_startsb"] = starts[MPC - 32 :].reshape(32, 1).copy()
                m["scl"][MPC - 32 : MPC, so["lensb"]] = lens[MPC - 32 :]
                starts = starts.copy()
                lens = lens.copy()
                starts[MPC - 32 :] = 0
                lens[MPC - 32 :] = 0.0
            m[name + "_starts"] = starts.reshape(MPC, 1)
            m["scl"][0:MPC, so["lens"]] = lens
        in_maps.append(m)
    return cfg, split_late, in_maps


def kernel(**inputs) -> np.ndarray:
    from concourse import bass_utils

    cfg, split_late, in_maps = _prep(inputs)
    nc = _get_nc(cfg, split_late)
    res = bass_utils.run_bass_kernel_spmd(nc, in_maps, core_ids=list(range(NCORES)))
    return np.concatenate([res.results[c]["out"] for c in range(NCORES)], axis=0)


# revision 41
# speedup vs baseline: 1.0794x; 1.0360x over previous
"""Trainium2 Bass kernel for BANDNN-style packed-molecule energy model.

Reference computation: four per-row MLPs (one per interaction type), each
row's scalar energy segment-summed per molecule; four per-molecule energies
added -> (512, 1).

Strategy (8 NeuronCores, data-parallel over molecules):
 - Each core owns 64 contiguous molecules; host slices the packed rows of all
   four types accordingly, transposes to feature-major layout (row-tiles
   folded 4-up across 32-partition strips), and zero-pads so all cores share
   one NEFF.
 - Per 512-row tile the MLP runs feature-major on the PE (h^T = W^T @ x^T).
   Matmul operands are bf16 (PE full rate; fp32r streams at half rate on
   TRN2); PSUM accumulation and everything downstream of the last matmul
   (e = h3.W4 + b4, segment sums) stays fp32. Final rel err vs the fp32
   reference is ~5e-3 (KERNEL_MM_DTYPE=f32r gives ~1e-4 at ~2x the time).
 - Emission is layer-major over groups of 8 tiles so the PE sees dense
   same-layer matmul bursts (keeps the HAM clock gate at 2.4 GHz; per-tile
   emission leaves micro-gaps that pin the PE at the cold 1.2 GHz rate).
   L1 uses 4-way tile_position row packing (K = 6..25); the four L4s of
   consecutive tiles write disjoint partition rows of one PSUM bank via
   shifted-column W4 copies, batching the energy-row copy 16 tiles at a
   time.
 - Per-row energies land in DRAM; a per-molecule indirect-DMA gather +
   iota<len mask + multiply-reduce produces the 64 per-molecule sums on each
   core; host concatenates the 8 outputs. For the last (largest) type the
   final 32 molecules gather separately so the main gather clears before the
   last chunk finishes.
"""

import os as _os
import sys

if "/opt/trn_rl_repo" not in sys.path:
    sys.path.insert(0, "/opt/trn_rl_repo")

import numpy as np

B = 512            # molecules
NCORES = 8
MPC = B // NCORES  # molecules per core
TILE = 512         # matmul moving-operand width (one fp32 PSUM bank)
CHW = 8192         # x^T DMA chunk width (16 tiles; also the L4/est batch)

# dihedrals last: the final type's phase-B chain (gather width RM) is the
# kernel tail, and dihedrals has a small RM; nonbonds' big gather overlaps
# dihedrals' compute.
TYPES = (("bonds", 13), ("angles", 19), ("nonbonds", 6), ("dihedrals", 25))

IOTA_W = 512

MM_DTYPE = _os.environ.get("KERNEL_MM_DTYPE", "bf16")  # "bf16" | "f32r"



def _round_up(x, m):
    return -(-x // m) * m


def _round_fp32r(a):
    """Round fp32 to float32r precision (11 explicit mantissa bits)."""
    u = np.ascontiguousarray(a, dtype=np.float32).view(np.uint32).astype(np.uint64)
    u = (u + 0x7FF + ((u >> 12) & 1)) >> 12 << 12
    return np.minimum(u, 0xFFFFFFFF).astype(np.uint32).view(np.float32)


def _const_layout(cfg):
    """Column layouts: weights tensor (matmul dtype) and scalars tensor (f32)."""
    woff = {}
    soff = {}
    wc = 0
    sc = 0
    for name, _F, _NP, _RM in cfg:
        d = {}
        for w in ("w1", "w2a", "w2b", "w3a", "w3b"):
            d[w] = wc
            wc += 128
        d["w4"] = wc
        wc += 1
        d["w4p"] = wc
        wc += 16 * 16
        woff[name] = d
        s = {}
        for k in ("b1", "b2a", "b2b", "b3", "b4", "lens", "lensb"):
            s[k] = sc
            sc += 1
        soff[name] = s
    soff["iota"] = sc
    sc += IOTA_W
    return woff, wc, soff, sc


def _build(cfg, split_late):
    """Build + bacc-compile the Bass kernel for the given per-type config."""
    from contextlib import ExitStack

    import concourse.bacc as bacc
    import concourse.mybir as mybir
    import concourse.tile as tile
    from concourse.bass import IndirectOffsetOnAxis
    from concourse.tile import add_dep_helper

    f32 = mybir.dt.float32
    i32 = mybir.dt.int32
    mmdt = mybir.dt.bfloat16 if MM_DTYPE == "bf16" else mybir.dt.float32r
    RELU = mybir.ActivationFunctionType.Relu
    IDENT = mybir.ActivationFunctionType.Identity
    ADD = mybir.AluOpType.add
    MAX = mybir.AluOpType.max

    woff, nw, soff, ns = _const_layout(cfg)

    nc = bacc.Bacc(
        "TRN2",
        target_bir_lowering=False,
        debug=False,
        enable_asserts=False,
        num_devices=NCORES,
    )

    wts_ap = nc.dram_tensor("wts", (128, nw), mmdt, kind="ExternalInput").ap()
    scl_ap = nc.dram_tensor("scl", (128, ns), f32, kind="ExternalInput").ap()
    xt_aps = {}
    ebuf_aps = {}
    starts_aps = {}
    for name, F, NP, _RM in cfg:
        xt_aps[name] = nc.dram_tensor(
            f"{name}_xt", (128, NP // 4), mmdt, kind="ExternalInput"
        ).ap()
        ebuf_aps[name] = nc.dram_tensor(
            f"{name}_ebuf", (NP, 1), f32, kind="Internal"
        ).ap()
        starts_aps[name] = nc.dram_tensor(
            f"{name}_starts", (MPC, 1), i32, kind="ExternalInput"
        ).ap()
    last_name = cfg[-1][0]
    if split_late:
        startsb_ap = nc.dram_tensor(
            f"{last_name}_startsb", (32, 1), i32, kind="ExternalInput"
        ).ap()
    out_ap = nc.dram_tensor("out", (MPC, 1), f32, kind="ExternalOutput").ap()

    with tile.TileContext(nc) as tc, ExitStack() as ctx:
        cpool = ctx.enter_context(tc.tile_pool(name="consts", bufs=1))
        warmt = cpool.tile([128, TILE], mmdt, tag="warm")
        nc.vector.memset(warmt[:], 0.125)
        wtile = cpool.tile([128, nw], mmdt, tag="wts")
        nc.sync.dma_start(wtile[:], wts_ap[:])
        stile = cpool.tile([128, ns], f32, tag="scl")
        nc.sync.dma_start(stile[:], scl_ap[:])

        xpools = {
            name: ctx.enter_context(
                tc.tile_pool(name=f"x_{name}", bufs=min(-(-NP // CHW), 4))
            )
            for name, _F, NP, _RM in cfg
        }
        h1p = ctx.enter_context(tc.tile_pool(name="h1", bufs=9))
        h2ap = ctx.enter_context(tc.tile_pool(name="h2a", bufs=9))
        h2bp = ctx.enter_context(tc.tile_pool(name="h2b", bufs=9))
        h3p = ctx.enter_context(tc.tile_pool(name="h3", bufs=9))
        ep = ctx.enter_context(tc.tile_pool(name="estage", bufs=2))
        ph13 = ctx.enter_context(tc.tile_pool(name="ph13", bufs=4, space="PSUM"))
        ph2 = ctx.enter_context(tc.tile_pool(name="ph2", bufs=3, space="PSUM"))
        pep = ctx.enter_context(tc.tile_pool(name="pe", bufs=1, space="PSUM"))
        segp = ctx.enter_context(tc.tile_pool(name="seg", bufs=1))

        # HAM warmup: ~24 dense back-to-back matmuls (~10us cold) trip the
        # PE clock gate to 8/8 before the real pipeline (which has micro-gaps
        # that never satisfy the "sustained busy" window) begins.
        warm = pep.tile([8, TILE], f32, tag="pe")
        for k in range(16):
            nc.tensor.matmul(
                warm[0:8, :], warmt[:, 0:8], warmt[:, :],
                start=(k == 0), stop=(k == 15),
            )

        masks = {}
        starts_tiles = {}
        for name, _F, _NP, RM in cfg:
            so = soff[name]
            st = segp.tile([MPC, 1], i32, tag=f"starts_{name}")
            nc.sync.dma_start(st[:], starts_aps[name][:])
            starts_tiles[name] = st
            mk = segp.tile([MPC, RM], f32, tag=f"mask_{name}")
            nc.vector.tensor_scalar(
                mk[:],
                stile[0:MPC, soff["iota"] : soff["iota"] + RM],
                stile[0:MPC, so["lens"] : so["lens"] + 1],
                None,
                op0=mybir.AluOpType.is_lt,
            )
            masks[name] = mk
            if split_late and name == last_name:
                mkb = segp.tile([32, RM], f32, tag="maskb")
                nc.vector.tensor_scalar(
                    mkb[:],
                    stile[MPC - 32 : MPC, soff["iota"] : soff["iota"] + RM],
                    stile[MPC - 32 : MPC, so["lensb"] : so["lensb"] + 1],
                    None,
                    op0=mybir.AluOpType.is_lt,
                )
                masks["_late"] = mkb

        acc = None
        for name, F, NP, RM in cfg:
            wo = woff[name]
            so = soff[name]
            w1g = [
                wtile[32 * g : 32 * g + F, wo["w1"] : wo["w1"] + 128]
                for g in range(4)
            ]
            w2a = wtile[:, wo["w2a"] : wo["w2a"] + 128]
            w2b = wtile[:, wo["w2b"] : wo["w2b"] + 128]
            w3a = wtile[:, wo["w3a"] : wo["w3a"] + 128]
            w3b = wtile[:, wo["w3b"] : wo["w3b"] + 128]
            w4 = wtile[:, wo["w4"] : wo["w4"] + 1]
            b1 = stile[:, so["b1"] : so["b1"] + 1]
            b2a = stile[:, so["b2a"] : so["b2a"] + 1]
            b2b = stile[:, so["b2b"] : so["b2b"] + 1]
            b3 = stile[:, so["b3"] : so["b3"] + 1]
            b4 = stile[:, so["b4"] : so["b4"] + 1]
            xt_ap = xt_aps[name]
            ebuf_ap = ebuf_aps[name]
            ebuf_writes = []

            chunk_plan = [(c0, min(CHW, NP - c0)) for c0 in range(0, NP, CHW)]
            for c0, w in chunk_plan:
                nt = w // TILE
                xt = xpools[name].tile([128, w // 4], mmdt)
                nc.sync.dma_start(xt[:], xt_ap[:, c0 // 4 : (c0 + w) // 4])
                est = ep.tile([nt, TILE], f32, tag="est")
                # one PSUM bank collects the chunk's nt (1,512) energy rows
                pe = pep.tile([16, TILE], f32, tag="pe")
                # layer-major over groups of 4 tiles: PE runs 4+ same-layer
                # matmuls back-to-back while ACT/DVE copies chase, instead of
                # stalling on each tile's copy chain.
                for g0 in range(0, nt, 8):
                    gn = min(8, nt - g0)
                    p1s = []
                    h1s = []
                    for sub0 in range(0, gn, 4):
                        sn = min(4, gn - sub0)
                        qq = (g0 + sub0) // 4
                        subp = []
                        for g in range(sn):
                            rhs = xt[32 * g : 32 * g + F, qq * TILE : (qq + 1) * TILE]
                            p1 = ph13.tile([128, TILE], f32, tag="p13")
                            nc.tensor.matmul(
                                p1[:], w1g[g], rhs, start=True, stop=True,
                                tile_position=(32 * g, 0),
                            )
                            subp.append(p1)
                        for g in range(sn):
                            h1 = h1p.tile([128, TILE], mmdt)
                            nc.scalar.activation(
                                h1[:], subp[g][:], RELU, bias=b1, scale=1.0
                            )
                            h1s.append(h1)
                    h2s = []
                    for k in range(gn):
                        p2a = ph2.tile([128, TILE], f32, tag="p2")
                        nc.tensor.matmul(p2a[:], w2a, h1s[k][:], start=True, stop=True)
                        h2a = h2ap.tile([128, TILE], mmdt)
                        nc.vector.tensor_scalar(h2a[:], p2a[:], b2a, 0.0, op0=ADD, op1=MAX)
                        p2b = ph2.tile([128, TILE], f32, tag="p2")
                        nc.tensor.matmul(p2b[:], w2b, h1s[k][:], start=True, stop=True)
                        h2b = h2bp.tile([128, TILE], mmdt)
                        nc.vector.tensor_scalar(h2b[:], p2b[:], b2b, 0.0, op0=ADD, op1=MAX)
                        h2s.append((h2a, h2b))
                    h3s = []
                    for k in range(gn):
                        h2a, h2b = h2s[k]
                        p3 = ph13.tile([128, TILE], f32, tag="p13")
                        nc.tensor.matmul(p3[:], w3a, h2a[:], start=True, stop=False)
                        nc.tensor.matmul(p3[:], w3b, h2b[:], start=False, stop=True)
                        h3 = h3p.tile([128, TILE], mmdt)
                        nc.scalar.activation(h3[:], p3[:], RELU, bias=b3, scale=1.0)
                        h3s.append(h3)
                    for k in range(gn):
                        j = g0 + k
                        w4p = wtile[:, wo["w4p"] + 16 * j : wo["w4p"] + 16 * (j + 1)]
                        nc.tensor.matmul(
                            pe[0:16, :],
                            w4p,
                            h3s[k][:],
                            start=(j == 0),
                            stop=(j == nt - 1),
                        )
                nc.scalar.activation(
                    est[:, :], pe[0:nt, :], IDENT, bias=b4[0:nt, :], scale=1.0
                )
                ebuf_writes.append(
                    nc.sync.dma_start(ebuf_ap[c0 : c0 + w, :], est[:]).ins
                )

            # ---- per-molecule segment reduction for this type ----
            # For the last type, molecules 32-63 (whose gather window can
            # touch the final chunk) are gathered separately so the main
            # gather only waits on all-but-the-last chunk.
            split = split_late and name == last_name
            starts_sb = starts_tiles[name]
            mask = masks[name]
            eg = segp.tile([MPC, RM], f32, tag=f"eg_{name}")
            g = nc.gpsimd.indirect_dma_start(
                out=eg[:],
                out_offset=None,
                in_=ebuf_ap[:, :],
                in_offset=IndirectOffsetOnAxis(ap=starts_sb[:, :1], axis=0),
            )
            for winst in ebuf_writes[: -1 if split else None]:
                add_dep_helper(g.ins, winst, sync=True, reason="ebuf RAW")

            scratch = segp.tile([MPC, RM], f32, tag=f"scr_{name}")
            acc_t = segp.tile([MPC, 1], f32, tag=f"acct_{name}")
            nc.vector.tensor_tensor(
                out=scratch[:], in0=eg[:], in1=mask[:], op=mybir.AluOpType.mult
            )
            nc.vector.reduce_sum(acc_t[:], scratch[:], axis=mybir.AxisListType.X)
            if acc is None:
                acc = acc_t
            else:
                acc_new = segp.tile([MPC, 1], f32, tag=f"acc_{name}")
                nc.vector.tensor_tensor(
                    out=acc_new[:], in0=acc[:], in1=acc_t[:], op=ADD
                )
                acc = acc_new
            if split:
                startsb_sb = segp.tile([32, 1], i32, tag="startsb")
                nc.sync.dma_start(startsb_sb[:], startsb_ap[:])
                egb = segp.tile([32, RM], f32, tag="egb")
                gb = nc.gpsimd.indirect_dma_start(
                    out=egb[:],
                    out_offset=None,
                    in_=ebuf_ap[:, :],
                    in_offset=IndirectOffsetOnAxis(ap=startsb_sb[:, :1], axis=0),
                )
                for winst in ebuf_writes:
                    add_dep_helper(gb.ins, winst, sync=True, reason="ebuf RAW b")
                scrb = segp.tile([32, RM], f32, tag="scrb")
                accb = segp.tile([MPC, 1], f32, tag="accb")
                nc.vector.memset(accb[:], 0.0)
                nc.vector.tensor_tensor(
                    out=scrb[:], in0=egb[:], in1=masks["_late"][:, :],
                    op=mybir.AluOpType.mult,
                )
                nc.vector.reduce_sum(
                    accb[MPC - 32 : MPC, :], scrb[:], axis=mybir.AxisListType.X
                )
                acc_fin = segp.tile([MPC, 1], f32, tag="accfin")
                nc.vector.tensor_tensor(
                    out=acc_fin[:], in0=acc[:], in1=accb[:], op=ADD
                )
                acc = acc_fin

        nc.sync.dma_start(out_ap[:, :], acc[:])

    nc.compile()
    return nc


_CACHE = {}


def _get_nc(cfg, split_late):
    key = (cfg, split_late)
    if key not in _CACHE:
        _CACHE[key] = _build(cfg, split_late)
    return _CACHE[key]


def _mm_np_dtype():
    if MM_DTYPE == "bf16":
        import ml_dtypes

        return np.dtype(ml_dtypes.bfloat16)
    return np.dtype(np.float32)


def _to_mm(a):
    if MM_DTYPE == "bf16":
        return np.ascontiguousarray(a).astype(_mm_np_dtype())
    return _round_fp32r(a)


def _prep(inputs):
    """Host-side sharding: per-core input dicts + per-type config."""
    cfg = []
    per_type = {}
    for name, F in TYPES:
        x = np.ascontiguousarray(np.asarray(inputs[name + "_x"], dtype=np.float32))
        seg = np.asarray(inputs[name + "_seg"], dtype=np.int64)
        assert x.shape[1] == F
        counts = np.bincount(seg, minlength=B)
        bounds = np.searchsorted(seg, np.arange(0, B + 1, MPC))
        rows_pc = np.diff(bounds)
        RM = _round_up(int(counts.max()), 4)
        assert RM <= IOTA_W
        NP = _round_up(int(rows_pc.max()) + RM, 4 * TILE)
        mol_starts = np.searchsorted(seg, np.arange(B))
        per_type[name] = dict(
            x=x, counts=counts, bounds=bounds, mol_starts=mol_starts, NP=NP, RM=RM
        )
        cfg.append((name, F, NP, RM))
    cfg = tuple(cfg)

    woff, nw, soff, ns = _const_layout(cfg)
    mmdt = _mm_np_dtype()

    # late-molecule split eligibility for the last type: molecule MPC-32 of
    # every core must start early enough that its gather window stays clear
    # of the final chunk
    lname = TYPES[-1][0]
    lpt = per_type[lname]
    last_w = lpt["NP"] - ((lpt["NP"] - 1) // CHW) * CHW
    cut = lpt["NP"] - last_w
    split_late = True
    for c in range(NCORES):
        s = int(lpt["bounds"][c])
        st32 = int(lpt["mol_starts"][c * MPC + MPC - 32] - s)
        if st32 + lpt["RM"] > cut:
            split_late = False

    wts = np.zeros((128, nw), dtype=mmdt)
    scl = np.zeros((128, ns), dtype=np.float32)
    for name, F in TYPES:
        wo = woff[name]
        so = soff[name]
        params = inputs[name + "_params"]
        (W1, b1), (W2, b2), (W3, b3), (W4, b4) = [
            (np.asarray(w, np.float32), np.asarray(b, np.float32)) for w, b in params
        ]
        for g in range(4):
            wts[32 * g : 32 * g + F, wo["w1"] : wo["w1"] + 128] = _to_mm(W1)
        wts[:, wo["w2a"] : wo["w2a"] + 128] = _to_mm(W2[:, :128])
        wts[:, wo["w2b"] : wo["w2b"] + 128] = _to_mm(W2[:, 128:])
        wts[:, wo["w3a"] : wo["w3a"] + 128] = _to_mm(W3[:128, :])
        wts[:, wo["w3b"] : wo["w3b"] + 128] = _to_mm(W3[128:, :])
        wts[:, wo["w4"]] = _to_mm(W4[:, 0])
        for j in range(16):
            wts[:, wo["w4p"] + 16 * j + j] = _to_mm(W4[:, 0])
        scl[:, so["b1"]] = b1
        scl[:, so["b2a"]] = b2[:128]
        scl[:, so["b2b"]] = b2[128:]
        scl[:, so["b3"]] = b3
        scl[:, so["b4"]] = b4[0]
    scl[0:MPC, soff["iota"] : soff["iota"] + IOTA_W] = np.arange(
        IOTA_W, dtype=np.float32
    )[None, :]

    in_maps = []
    for c in range(NCORES):
        m = {"wts": wts, "scl": scl.copy()}
        for name, F in TYPES:
            pt = per_type[name]
            s, e = int(pt["bounds"][c]), int(pt["bounds"][c + 1])
            NPt = pt["NP"]
            xT = np.zeros((F, NPt), dtype=np.float32)
            xT[:, : e - s] = pt["x"][s:e].T
            x3 = xT.reshape(F, NPt // (4 * TILE), 4, TILE)
            fold = np.zeros((128, NPt // 4), dtype=mmdt)
            for g in range(4):
                fold[32 * g : 32 * g + F] = _to_mm(
                    x3[:, :, g, :].reshape(F, NPt // 4)
                )
            m[name + "_xt"] = fold
            so = soff[name]
            starts = (pt["mol_starts"][c * MPC : (c + 1) * MPC] - s).astype(np.int32)
            lens = pt["counts"][c * MPC : (c + 1) * MPC].astype(np.float32)
            if split_late and name == TYPES[-1][0]:
                m[name + "_startsb"] = starts[MPC - 32 :].reshape(32, 1).copy()
                m["scl"][MPC - 32 : MPC, so["lensb"]] = lens[MPC - 32 :]
                starts = starts.copy()
                lens = lens.copy()
                starts[MPC - 32 :] = 0
                lens[MPC - 32 :] = 0.0
            m[name + "_starts"] = starts.reshape(MPC, 1)
            m["scl"][0:MPC, so["lens"]] = lens
        in_maps.append(m)
    return cfg, split_late, in_maps


def kernel(**inputs) -> np.ndarray:
    from concourse import bass_utils

    cfg, split_late, in_maps = _prep(inputs)
    nc = _get_nc(cfg, split_late)
    res = bass_utils.run_bass_kernel_spmd(nc, in_maps, core_ids=list(range(NCORES)))
    return np.concatenate([res.results[c]["out"] for c in range(NCORES)], axis=0)
